# revision 15
# baseline (speedup 1.0000x reference)
"""Chunked cross-attention (RETRO-style) Trainium2 Bass kernel.

Contract: kernel(**inputs) takes FULL unsharded inputs (as produced by the
problem's setup_inputs) and returns the FULL [4, 2048, 1024] f32 output.

Sharding: data-parallel over (batch, chunk-half). Core i handles batch i//2,
chunks [16*(i%2), 16*(i%2)+16). Each core is fully independent (no
collectives). Host folds ln_g/ln_b into Wq/bq, casts e + weights, slices h/e
per core, and stitches the 8 per-core outputs back together.

v3 attention layout: scores are computed TRANSPOSED (k stationary, q moving
-> sT[kv, tok]), so exp's PSUM->SBUF activation IS the probs copy and no PE
transposes are needed. Softmax denominators ride along as 64 ones-columns
appended to each head's V slice: attnV produces [out^T; denom*64] in one
accumulation, then a 64-partition DVE reciprocal + multiply normalizes and
writes oT directly. The whole Q projection is batched into the prologue as
dense N=512 DoubleRow matmuls. K and V live in fp8 (they are matmul
stationary operands; 1/64 descales are folded into the exp scale and the
final output scale).
"""

import os
import sys

sys.path.insert(0, "/opt/trn_rl_repo")

from contextlib import ExitStack

import numpy as np
import ml_dtypes

import concourse.bass as bass
import concourse.bacc as bacc
import concourse.mybir as mybir
import concourse.tile as tile
from concourse.bass_utils import run_bass_kernel_spmd

P = 128
D = 1024
H = 16
DK = 64
L = 64
ITERS = 8  # 2 chunks per iteration, 16 chunks per core
EPS = 1e-5
SCALE = 1.0 / 8.0  # 1/sqrt(DK)

F32 = mybir.dt.float32
BF16 = mybir.dt.bfloat16
FP8 = mybir.dt.float8e4
F8 = ml_dtypes.float8_e4m3
VSCALE = 64.0  # weights pre-scaled by this on host (fp8 subnormal dodge)
BF = ml_dtypes.bfloat16

LAST_EXEC_NS = None
LAST_RESULTS = None


def build_nc(with_bq, with_bk, with_bv, with_bo):
    nc = bacc.Bacc("TRN2", target_bir_lowering=False, debug=False)

    h_s = nc.dram_tensor("h_s", [ITERS * P, D], F32, kind="ExternalInput")
    e_s = nc.dram_tensor("e_s", [ITERS * 512, D], BF16, kind="ExternalInput")
    wq_d = nc.dram_tensor("wq", [D, D], FP8, kind="ExternalInput")
    wk_d = nc.dram_tensor("wk", [D, D], FP8, kind="ExternalInput")
    wv_d = nc.dram_tensor("wv", [D, D], FP8, kind="ExternalInput")
    wo_d = nc.dram_tensor("wo", [D, D], FP8, kind="ExternalInput")
    bq_d = nc.dram_tensor("bq", [1, D], F32, kind="ExternalInput")
    bk_d = nc.dram_tensor("bk", [1, D], F32, kind="ExternalInput")
    bv_d = nc.dram_tensor("bv", [1, D], F32, kind="ExternalInput")
    bo_d = nc.dram_tensor("bo", [1, D], F32, kind="ExternalInput")
    out_s = nc.dram_tensor("out_s", [ITERS * P, D], F32, kind="ExternalOutput")

    Exp = mybir.ActivationFunctionType.Exp
    Square = mybir.ActivationFunctionType.Square
    X = mybir.AxisListType.X
    DR = mybir.MatmulPerfMode.DoubleRow

    with tile.TileContext(nc) as tc, ExitStack() as ctx:
        consts = ctx.enter_context(tc.tile_pool(name="consts", bufs=1))
        ones = consts.tile([1, 512], F32)
        nc.vector.memset(ones, 1.0)
        ones64 = consts.tile([P, 64], BF16)
        nc.vector.memset(ones64, 1.0)

        # weight tiles (DMAs emitted below in consumer-priority order)
        wk_t = consts.tile([P, 4, 2, D], FP8)
        wq_t = consts.tile([P, 4, 2, D], FP8)
        wv_t = consts.tile([P, 4, 2, D], FP8)
        wo_t = consts.tile([P, 4, 2, D], FP8)

        bq_t = bk_t = bv_t = bo_t = None
        if with_bq:
            bq_t = consts.tile([1, D], F32, name="bq_t")
            nc.sync.dma_start(bq_t, bq_d)
        if with_bk:
            bk_t = consts.tile([1, D], F32, name="bk_t")
            nc.sync.dma_start(bk_t, bk_d)
        if with_bv:
            bv_t = consts.tile([1, D], F32, name="bv_t")
            nc.sync.dma_start(bv_t, bv_d)
        if with_bo:
            bo_t = consts.tile([1, D], F32, name="bo_t")
            nc.sync.dma_start(bo_t, bo_d)

        res = ctx.enter_context(tc.tile_pool(name="res", bufs=1))
        sb = ctx.enter_context(tc.tile_pool(name="sb", bufs=2))
        hd = ctx.enter_context(tc.tile_pool(name="hd", bufs=6))
        psA = ctx.enter_context(tc.tile_pool(name="psA", bufs=3, space="PSUM"))
        psB = ctx.enter_context(tc.tile_pool(name="psB", bufs=3, space="PSUM"))
        psS = ctx.enter_context(tc.tile_pool(name="psS", bufs=2, space="PSUM"))

        # PE warmup: dummy matmuls so HAM un-throttles the clock before the
        # real work arrives (the initial DMA wait would otherwise be cold).
        warm = consts.tile([P, 512], BF16, name="warm")
        nc.vector.memset(warm, 0.0)
        wp = psB.tile([P, 512], F32, name="m")
        for i in range(72):
            nc.tensor.matmul(wp, warm[:, 0:P], warm, start=(i == 0),
                             stop=(i == 71))
        warm_out = consts.tile([P, 512], BF16, name="warm_out")
        nc.vector.tensor_copy(warm_out, wp)

        hx_all = res.tile([P, ITERS, D], F32)
        stats = res.tile([P, ITERS, 8], F32)
        # q^T for the whole shard: [dk-pair partitions, head-pair, token]
        qT_all = res.tile([P, 8, ITERS * P], BF16)
        xT8_all = res.tile([P, 8, ITERS * P], FP8)
        # v: [kv, chunk*kvhalf, head, dk], manually double-buffered
        v128s = [res.tile([P, 4, H, 64], FP8, name=f"v128_{i}")
                 for i in range(2)]

        # DMA emission order = scheduler priority. Get iter-0's operands in
        # first (eT0 + wk -> k^T, h + wq -> LN -> q^T, wv -> v), then the
        # bulk loads.
        eTs = [sb.tile([P, 8, 512], BF16, name="eT") for _ in range(ITERS)]
        nc.sync.dma_start(eTs[0], e_s[0:512, :], transpose=True)
        nc.sync.dma_start(wk_t, wk_d.rearrange("(kp h p) m -> p kp h m", p=P, h=2))
        nc.sync.dma_start(hx_all[:, 0, :], h_s[0:P, :])
        nc.sync.dma_start(wq_t, wq_d.rearrange("(kp h p) m -> p kp h m", p=P, h=2))
        nc.sync.dma_start(wv_t, wv_d.rearrange("(kp h p) m -> p kp h m", p=P, h=2))
        for it in range(1, ITERS):
            nc.sync.dma_start(hx_all[:, it, :], h_s[it * P:(it + 1) * P, :])
        nc.sync.dma_start(wo_t, wo_d.rearrange("(kp h p) m -> p kp h m", p=P, h=2))
        nc.sync.dma_start(eTs[1], e_s[512:1024, :], transpose=True)

        # ===== prologue: LN for all 8 iterations =====
        for it in range(ITERS):
            hx = hx_all[:, it, :]
            ssum = stats[:, it, 0:1]
            ssq = stats[:, it, 1:2]
            negmu = stats[:, it, 2:3]
            musq = stats[:, it, 3:4]
            var = stats[:, it, 4:5]
            nc.vector.reduce_sum(ssum, hx, axis=X)
            sqscr = sb.tile([P, D], BF16, name="sqscr")
            nc.scalar.activation(sqscr, hx, Square, accum_out=ssq)
            nc.vector.tensor_scalar_mul(negmu, ssum, -1.0 / D)
            nc.vector.tensor_mul(musq, negmu, negmu)
            nc.vector.tensor_scalar(var, ssq, 1.0 / D, EPS,
                                    op0=mybir.AluOpType.mult,
                                    op1=mybir.AluOpType.add)
            nc.vector.tensor_sub(var, var, musq)
        # batched sqrt (few ACT table loads) + reciprocal, split so the
        # first iterations' x_hat unblocks before all stats are in
        nc.scalar.sqrt(stats[:, 0:2, 5:6], stats[:, 0:2, 4:5])
        nc.vector.reciprocal(stats[:, 0:2, 6:7], stats[:, 0:2, 5:6])
        nc.scalar.sqrt(stats[:, 2:, 5:6], stats[:, 2:, 4:5])
        nc.vector.reciprocal(stats[:, 2:, 6:7], stats[:, 2:, 5:6])
        for it in range(ITERS):
            hx = hx_all[:, it, :]
            negmu = stats[:, it, 2:3]
            rstd = stats[:, it, 6:7]
            xh = sb.tile([P, D], BF16, name="xh")
            nc.vector.tensor_scalar(xh, hx, negmu, rstd,
                                    op0=mybir.AluOpType.add,
                                    op1=mybir.AluOpType.mult)
            xT = sb.tile([P, 8, P], BF16, name="xT")
            nc.sync.dma_start(xT, xh, transpose=True)
            nc.vector.tensor_copy(xT8_all[:, :, it * P:(it + 1) * P], xT)

        # ===== prologue: q^T for the whole shard, dense N=512 matmuls =====
        # qT_all holds 64*q (descale folded into the exp scale).
        for m in range(8):
            for half in range(2):
                pq = psA.tile([P, 512], F32, name="t")
                for kp in range(4):
                    nc.tensor.matmul(pq, wq_t[:, kp, :, m * P:(m + 1) * P],
                                     xT8_all[:, 2 * kp:2 * kp + 2,
                                             half * 512:(half + 1) * 512],
                                     start=(kp == 0),
                                     stop=(kp == 3 and not with_bq),
                                     perf_mode=DR)
                if with_bq:
                    nc.tensor.matmul(pq, bq_t[0:1, m * P:(m + 1) * P],
                                     ones[0:1, 0:512], start=False, stop=True)
                nc.vector.tensor_copy(
                    qT_all[:, m, half * 512:(half + 1) * 512], pq)

        # ===== k^T / v projections (phase B bodies) =====
        kT8s = {}

        def emit_proj(it):
            """All of iteration it's k^T and v projections, densely."""
            eT = eTs[it]
            eT8 = sb.tile([P, 8, 512], FP8, name="eT8")
            nc.vector.tensor_copy(eT8[:, 0:4, :], eT[:, 0:4, :])
            nc.vector.tensor_copy(eT8[:, 4:8, :], eT[:, 4:8, :])
            kT8s[it] = sb.tile([P, 8, 512], FP8, name="kT8")
            v128 = v128s[it % 2]
            for m in range(8):
                pk = psB.tile([P, 512], F32, name="m")
                for kp in range(4):
                    nc.tensor.matmul(pk, wk_t[:, kp, :, m * P:(m + 1) * P],
                                     eT8[:, 2 * kp:2 * kp + 2, :],
                                     start=(kp == 0),
                                     stop=(kp == 3 and not with_bk),
                                     perf_mode=DR)
                if with_bk:
                    nc.tensor.matmul(pk, bk_t[0:1, m * P:(m + 1) * P],
                                     ones[0:1, 0:512], start=False, stop=True)
                nc.vector.tensor_scalar_mul(kT8s[it][:, m, :], pk, 0.25)
            for part in range(8):
                t, nh = divmod(part, 2)
                pv = psB.tile([P, 512], F32, name="m")
                for kp in range(4):
                    nc.tensor.matmul(pv,
                                     eT8[:, 2 * kp:2 * kp + 2,
                                         t * P:(t + 1) * P],
                                     wv_t[:, kp, :, nh * 512:(nh + 1) * 512],
                                     start=(kp == 0),
                                     stop=(kp == 3 and not with_bv),
                                     perf_mode=DR)
                if with_bv:
                    nc.tensor.matmul(pv, ones[0:1, 0:P],
                                     bv_t[0:1, nh * 512:(nh + 1) * 512],
                                     start=False, stop=True)
                if nh == 0:
                    nc.vector.tensor_scalar_mul(
                        v128[:, t, nh * 8:(nh + 1) * 8, :], pv, 0.25)
                else:
                    nc.scalar.mul(
                        v128[:, t, nh * 8:(nh + 1) * 8, :], pv, 0.25)

        emit_proj(0)

        # ===== main loop: transposed attention + projection phase =====
        ESCALE = SCALE / (VSCALE * 16.0)  # descale 64q * 16k inside exp
        for it in range(ITERS):
            if it + 2 < ITERS:
                nc.sync.dma_start(eTs[it + 2],
                                  e_s[(it + 2) * 512:(it + 3) * 512, :],
                                  transpose=True)
            kT8 = kT8s[it]
            v128 = v128s[it % 2]
            oT = sb.tile([P, 4, 2, P], FP8, name="oT")

            # ---- phase A: attention for all 8 head pairs ----
            for hp in range(8):
                # scores transposed: sT[kv, tok] per (head, chunk, kv-half),
                # k stationary / q moving. One PSUM bank per head so the
                # row-group-tiled pairs never write the same bank.
                pscs = [psS.tile([P, 4, 64], F32, name="s") for _ in range(2)]
                for c in range(2):
                    for u in range(2):
                        for ph in range(2):
                            nc.tensor.matmul(
                                pscs[ph][:, 2 * c + u, :],
                                kT8[ph * 64:(ph + 1) * 64, hp,
                                    c * 256 + u * P:c * 256 + (u + 1) * P],
                                qT_all[ph * 64:(ph + 1) * 64, hp,
                                       it * P + c * 64:it * P + (c + 1) * 64],
                                start=True, stop=True)
                # exp is also the PSUM->SBUF probs move; no normalization
                # here (denominators come out of the attnV matmul).
                expT = hd.tile([P, 2, 4, 64], BF16, name="expT")
                nc.scalar.activation(expT[:, 0, :, :], pscs[0], Exp,
                                     scale=ESCALE)
                nc.scalar.activation(expT[:, 1, :, :], pscs[1], Exp,
                                     scale=ESCALE)
                # attnV: unnormalized out^T per head (partitions 0-63)
                pos = [psA.tile([P, 2, 64], F32, name="t") for _ in range(2)]
                for ph in range(2):
                    h_ = 2 * hp + ph
                    for c in range(2):
                        for u in range(2):
                            nc.tensor.matmul(
                                pos[ph][0:64, c, :],
                                v128[:, 2 * c + u, h_, :],
                                expT[:, ph, 2 * c + u, :],
                                start=(u == 0), stop=(u == 1))
                # softmax denominators: ones^T @ expT, per (head, chunk)
                pden = psA.tile([P, 2, 2, 64], F32, name="t")
                for ph in range(2):
                    for c in range(2):
                        for u in range(2):
                            nc.tensor.matmul(
                                pden[0:64, ph, c, :], ones64,
                                expT[:, ph, 2 * c + u, :],
                                start=(u == 0), stop=(u == 1))
                rcp = hd.tile([P, 2, 2, 64], F32, name="rcp")
                nc.vector.reciprocal(rcp[0:64, :, :, :], pden[0:64, :, :, :])
                for ph in range(2):
                    nc.vector.tensor_mul(
                        oT[ph * 64:(ph + 1) * 64, hp // 2, hp % 2, :],
                        pos[ph][0:64, :, :], rcp[0:64, ph, :, :])

            # ---- phase B: next iteration's projections, then output ----
            if it + 1 < ITERS:
                emit_proj(it + 1)

            # out = oT.T @ Wo (+bo) + h; oT is 64*out, wo is 64*Wo, so the
            # PSUM holds 4096*(out+bo) -> descale by 1/4096.
            outsb = sb.tile([P, D], F32, name="outsb")
            for nh in range(2):
                pf = psB.tile([P, 512], F32, name="m")
                for kp in range(4):
                    nc.tensor.matmul(pf, oT[:, kp, :, :],
                                     wo_t[:, kp, :, nh * 512:(nh + 1) * 512],
                                     start=(kp == 0),
                                     stop=(kp == 3 and not with_bo),
                                     perf_mode=DR)
                if with_bo:
                    nc.tensor.matmul(pf, ones[0:1, 0:P],
                                     bo_t[0:1, nh * 512:(nh + 1) * 512],
                                     start=False, stop=True)
                nc.vector.scalar_tensor_tensor(
                    outsb[:, nh * 512:(nh + 1) * 512], pf,
                    1.0 / (VSCALE * 16.0),
                    hx_all[:, it, nh * 512:(nh + 1) * 512],
                    op0=mybir.AluOpType.mult, op1=mybir.AluOpType.add)
            nc.sync.dma_start(out_s[it * P:(it + 1) * P, :], outsb)

    nc.compile()
    return nc


def make_in_maps(h, e, Wq, bq, Wk, bk, Wv, bv, Wo, bo, ln_g, ln_b):
    """Shard/cast host-side. Returns (in_maps, bias_flags)."""
    h = np.asarray(h, dtype=np.float32)
    e = np.asarray(e, dtype=np.float32)
    Wq = np.asarray(Wq, dtype=np.float32)
    Wk = np.asarray(Wk, dtype=np.float32)
    Wv = np.asarray(Wv, dtype=np.float32)
    Wo = np.asarray(Wo, dtype=np.float32)
    bq = np.asarray(bq, dtype=np.float32)
    bk = np.asarray(bk, dtype=np.float32)
    bv = np.asarray(bv, dtype=np.float32)
    bo = np.asarray(bo, dtype=np.float32)
    ln_g = np.asarray(ln_g, dtype=np.float32)
    ln_b = np.asarray(ln_b, dtype=np.float32)

    # Fold LN affine into the Q projection: q = x_hat@(g*Wq) + (b@Wq + bq)
    wq_eff = (ln_g[:, None] * Wq * 64.0).astype(F8)
    bq_eff = (ln_b @ Wq + bq).astype(np.float32)[None, :]
    wk_b = (Wk * 64.0).astype(F8)
    wv_b = (Wv * 64.0).astype(F8)
    wo_b = (Wo * 64.0).astype(F8)

    flags = (bool(np.any(bq_eff)), bool(np.any(bk)), bool(np.any(bv)),
             bool(np.any(bo)))

    B, S, _ = h.shape
    in_maps = []
    for core in range(8):
        b, half = divmod(core, 2)
        s0 = 1024 * half + (L - 1)
        h_sh = np.zeros((1024, D), np.float32)
        n = min(1024, S - s0)
        h_sh[:n] = h[b, s0:s0 + n]
        e_sh = np.ascontiguousarray(
            e[b, 16 * half:16 * half + 16].reshape(4096, D)).astype(BF)
        in_maps.append({
            "h_s": h_sh,
            "e_s": e_sh,
            "wq": wq_eff, "wk": wk_b, "wv": wv_b, "wo": wo_b,
            "bq": bq_eff * 64.0, "bk": bk[None, :] * 64.0,
            "bv": bv[None, :] * 64.0,
            "bo": bo[None, :] * 1024.0,
        })
    return in_maps, flags


def assemble(h, results):
    h = np.asarray(h, dtype=np.float32)
    out = np.empty_like(h)
    out[:, :L - 1] = h[:, :L - 1]
    for core in range(8):
        b, half = divmod(core, 2)
        shard = results[core]["out_s"]
        s0 = 1024 * half + (L - 1)
        n = min(1024, 2048 - s0)
        out[b, s0:s0 + n] = shard[:n]
    return out


def _enable_axon_trace():
    """The image lacks antenv.axon_hooks; synthesize it with the ctypes NTFF
    hook from trn_boot so run_bass_kernel_spmd(trace=True) works, and no-op
    the S3 artifact upload."""
    import types

    try:
        import antenv.axon_hooks  # noqa: F401
        have = True
    except ImportError:
        have = False
    if not have:
        if "/root/.axon_site" not in sys.path:
            sys.path.insert(0, "/root/.axon_site")
        from trn_agent_boot.trn_boot import _ntff_profile_via_ctypes

        hook = _ntff_profile_via_ctypes("/opt/axon/libaxon_pjrt.so")
        mod = types.ModuleType("antenv.axon_hooks")
        mod._hook = hook
        mod.get_axon_ntff_profile_hook = lambda: mod._hook
        mod.set_axon_ntff_profile_hook = lambda h: setattr(mod, "_hook", h)
        sys.modules["antenv.axon_hooks"] = mod
        import antenv
        antenv.axon_hooks = mod
    import concourse.bass_utils as bu
    bu.upload_artifacts = lambda tmpdir: "local://" + tmpdir


def kernel(**inputs):
    global LAST_EXEC_NS, LAST_RESULTS
    in_maps, flags = make_in_maps(**inputs)
    nc = build_nc(*flags)
    trace = bool(int(os.environ.get("KBENCH_TRACE", "0")))
    if trace:
        try:
            _enable_axon_trace()
        except Exception as exc:  # profiling is best-effort
            print(f"trace setup failed ({exc!r}); running untraced")
            trace = False
    res = run_bass_kernel_spmd(nc, in_maps, core_ids=list(range(8)),
                               trace=trace)
    LAST_EXEC_NS = res.exec_time_ns
    LAST_RESULTS = res
    return assemble(inputs["h"], res.results)
